# revision 14
# baseline (speedup 1.0000x reference)
"""Trainium2 Bass kernel for nn_Attn_Net_Gated (segment_reduce).

Computes, for feature [N, D] fp32 and sorted segment ids batch [N]:
  out1 = feature / max(||feature||_row, 1e-12)
  s    = (sigmoid(feature @ Wa.T + ba) * tanh(feature @ Wb.T + bb)) @ Wc.T + bc
  out2 = segment_softmax(s, batch)        (64 segments)

Sharding: rows split equally across 8 NeuronCores; segment denominators are
combined with a tiny [64]-float AllReduce, so segments may straddle shards.
"""

import os
import sys
import tempfile

import numpy as np

sys.path.insert(0, "/opt/trn_rl_repo")

import concourse.bass as bass
import concourse.bacc as bacc
import concourse.mybir as mybir
from concourse.tile import TileContext
from concourse.bass_utils import run_bass_kernel_spmd

N, D, L, NSEG, NCORES = 262144, 1024, 128, 64, 8
P = 128           # partitions
BLK = 4           # row-tiles per DMA block
F32 = mybir.dt.float32
BF16 = mybir.dt.bfloat16
I32 = mybir.dt.int32
AF = mybir.ActivationFunctionType
ALU = mybir.AluOpType

RSQRT_MAGIC = 0x5F3759DF


def build(rpc: int) -> bass.Bass:
    """Build the SPMD program for one core processing `rpc` rows."""
    T = rpc // P                    # row-tiles per core
    NB = T // BLK                   # DMA blocks per core
    G4 = 2                          # blocks per rsqrt batch
    assert T % BLK == 0 and NB % G4 == 0

    nc = bacc.Bacc(num_devices=NCORES)

    feat = nc.declare_dram_parameter("feat", [rpc, D], F32, isOutput=False)
    waT = nc.declare_dram_parameter("waT", [P, D], BF16, isOutput=False)
    wbT = nc.declare_dram_parameter("wbT", [P, D], BF16, isOutput=False)
    wc4 = nc.declare_dram_parameter("wc4", [P, BLK * L], BF16, isOutput=False)
    bias_ab = nc.declare_dram_parameter("bias_ab", [1, 2 * L], BF16, isOutput=False)
    ones1 = nc.declare_dram_parameter("ones1", [1, L], BF16, isOutput=False)
    ident = nc.declare_dram_parameter("ident", [P, P], BF16, isOutput=False)
    Rm = nc.declare_dram_parameter("Rm", [P, T * NSEG], BF16, isOutput=False)
    RTm = nc.declare_dram_parameter("RTm", [NSEG, T * P], BF16, isOutput=False)

    out_norm = nc.declare_dram_parameter("out_norm", [rpc, D], F32, isOutput=True)
    out_score = nc.declare_dram_parameter("out_score", [P, T], F32, isOutput=True)

    # collective bounce buffers (internal DRAM)
    cc_in = nc.dram_tensor("cc_in", [NSEG, 1], F32)
    cc_out = nc.dram_tensor("cc_out", [NSEG, 1], F32)

    feat_v = feat[:, :].rearrange("(t p) d -> t p d", p=P)
    onrm_v = out_norm[:, :].rearrange("(t p) d -> t p d", p=P)

    RCH = min(32, T)                # row-tiles per R-chunk DMA
    NCHUNK = 4 if (T // 4) % RCH == 0 else 1  # denominator-sum chunks
    with TileContext(nc) as tc:
        with (
            tc.tile_pool(name="const", bufs=1) as cpool,
            tc.tile_pool(name="fpool", bufs=6) as fpool,
            tc.tile_pool(name="f16pool", bufs=2) as f16pool,
            tc.tile_pool(name="ftpool", bufs=2) as ftpool,
            tc.tile_pool(name="scrpool", bufs=2) as scrpool,
            tc.tile_pool(name="actpool", bufs=2) as actpool,
            tc.tile_pool(name="nrpool", bufs=2) as nrpool,
            tc.tile_pool(name="tailpool", bufs=1) as tailpool,
            tc.tile_pool(name="rpool", bufs=2) as rpool,
            tc.tile_pool(name="pstr", bufs=2, space="PSUM") as pstr_pool,
            tc.tile_pool(name="psmm", bufs=2, space="PSUM") as psmm_pool,
            tc.tile_pool(name="pstail", bufs=1, space="PSUM") as pstail_pool,
        ):
            # ---- resident constants ----
            waT_sb = cpool.tile([P, D], BF16, tag="waT")
            nc.sync.dma_start(waT_sb[:, :], waT[:, :])
            wbT_sb = cpool.tile([P, D], BF16, tag="wbT")
            nc.sync.dma_start(wbT_sb[:, :], wbT[:, :])
            wc4_sb = cpool.tile([P, BLK * L], BF16, tag="wc4")
            nc.sync.dma_start(wc4_sb[:, :], wc4[:, :])
            bias_sb = cpool.tile([1, 2 * L], BF16, tag="bias")
            nc.sync.dma_start(bias_sb[:, :], bias_ab[:, :])
            ones_sb = cpool.tile([1, L], BF16, tag="ones")
            nc.sync.dma_start(ones_sb[:, :], ones1[:, :])
            ident_sb = cpool.tile([P, P], BF16, tag="ident")
            nc.sync.dma_start(ident_sb[:, :], ident[:, :])

            s_all = cpool.tile([P, T], F32, tag="s_all")
            ex16 = cpool.tile([P, T], BF16, tag="ex16")
            psD = pstail_pool.tile([NSEG, 1], F32, tag="psD")
            n2_all = cpool.tile([P, T], F32, tag="n2_all")
            inv_all = cpool.tile([P, T], F32, tag="inv_all")

            # ---- main loop over row blocks ----
            fblks = {}
            for b in range(NB):
                Fblk = fpool.tile([P, BLK * D], F32, tag="F")
                fblks[b] = Fblk
                nc.sync.dma_start(
                    Fblk[:, :].rearrange("p (i d) -> p i d", d=D),
                    feat_v[b * BLK:(b + 1) * BLK].rearrange("i p d -> p i d"),
                )
                F16 = f16pool.tile([P, BLK * D], BF16, tag="F16")
                fT16 = ftpool.tile([P, BLK * D], BF16, tag="fT16")
                a_blk = actpool.tile([P, BLK * L], BF16, tag="a")
                b_blk = actpool.tile([P, BLK * L], BF16, tag="b")

                for i in range(BLK):
                    t = b * BLK + i
                    Fi = Fblk[:, i * D:(i + 1) * D]

                    # row sum-of-squares (ACT square + accumulate)
                    sq_scr = scrpool.tile([P, D], BF16, tag="sq")
                    nc.scalar.activation(
                        sq_scr[:, :], Fi, AF.Square,
                        accum_out=n2_all[:, t:t + 1],
                    )

                    # cast fp32 -> bf16 for the matmul path
                    nc.vector.tensor_copy(F16[:, i * D:(i + 1) * D], Fi)

                    # transpose the bf16 tile: 8 PE transposes of [128,128]
                    ps_tr = pstr_pool.tile([P, D], BF16, tag="ps_tr")
                    for c in range(8):
                        nc.tensor.transpose(
                            ps_tr[:, c * P:(c + 1) * P],
                            F16[:, i * D + c * P: i * D + (c + 1) * P],
                            ident_sb[:, :],
                        )
                    nc.vector.tensor_copy(fT16[:, i * D:(i + 1) * D], ps_tr[:, :])

                    # gated-attention matmuls: psA = f @ Wa.T, psB = f @ Wb.T
                    psA = psmm_pool.tile([P, L], F32, tag="psA")
                    psB = psmm_pool.tile([P, L], F32, tag="psB")
                    for c in range(8):
                        fc = fT16[:, i * D + c * P: i * D + (c + 1) * P]
                        nc.tensor.matmul(
                            psA[:, :], fc, waT_sb[:, c * P:(c + 1) * P],
                            start=(c == 0), stop=False,
                        )
                        nc.tensor.matmul(
                            psB[:, :], fc, wbT_sb[:, c * P:(c + 1) * P],
                            start=(c == 0), stop=False,
                        )
                    # + bias row (ba | bb) via rank-1 matmul
                    nc.tensor.matmul(
                        psA[:, :], ones_sb[:, :], bias_sb[:, 0:L],
                        start=False, stop=True,
                    )
                    nc.tensor.matmul(
                        psB[:, :], ones_sb[:, :], bias_sb[:, L:2 * L],
                        start=False, stop=True,
                    )
                    nc.scalar.activation(a_blk[:, i * L:(i + 1) * L], psA[:, :], AF.Sigmoid)
                    nc.scalar.activation(b_blk[:, i * L:(i + 1) * L], psB[:, :], AF.Tanh)

                # s[r] = sum_l a*b*wc : two block-wide DVE mults,
                # then a per-tile ACT identity with accumulate
                bw_blk = actpool.tile([P, BLK * L], BF16, tag="bw")
                nc.vector.tensor_tensor(bw_blk[:, :], b_blk[:, :], wc4_sb[:, :], op=ALU.mult)
                g_blk = actpool.tile([P, BLK * L], BF16, tag="g")
                nc.vector.tensor_tensor(g_blk[:, :], a_blk[:, :], bw_blk[:, :], op=ALU.mult)
                for i in range(BLK):
                    t = b * BLK + i
                    scr_id = scrpool.tile([P, L], BF16, tag="scr_id")
                    nc.scalar.activation(
                        scr_id[:, :], g_blk[:, i * L:(i + 1) * L], AF.Identity,
                        accum_out=s_all[:, t:t + 1],
                    )

                if (b + 1) % (NB // NCHUNK) == 0:
                    # segment-denominator partial sums for the finished chunk
                    g = (b + 1) // (NB // NCHUNK) - 1
                    t0, t1 = g * (T // NCHUNK), (g + 1) * (T // NCHUNK)
                    nc.scalar.activation(ex16[:, t0:t1], s_all[:, t0:t1], AF.Exp)
                    for rg in range(t0 // RCH, t1 // RCH):
                        R_sb = rpool.tile([P, RCH * NSEG], BF16, tag="R")
                        nc.sync.dma_start(
                            R_sb[:, :], Rm[:, rg * RCH * NSEG:(rg + 1) * RCH * NSEG])
                        for j in range(RCH):
                            t = rg * RCH + j
                            nc.tensor.matmul(
                                psD[:, :],
                                R_sb[:, j * NSEG:(j + 1) * NSEG],
                                ex16[:, t:t + 1],
                                start=(t == 0), stop=(t == T - 1),
                            )

                if b % G4 == G4 - 1:
                    # rsqrt of n2 over the last G4 blocks:
                    # bit-hack seed + 2 Newton steps (rel err ~5e-6)
                    c0 = (b - G4 + 1) * BLK
                    n2b = n2_all[:, c0:c0 + G4 * BLK]
                    invb = inv_all[:, c0:c0 + G4 * BLK]
                    invb_i = invb.bitcast(I32)
                    nc.vector.tensor_scalar(
                        invb_i, n2b.bitcast(I32), 1, None, op0=ALU.logical_shift_right)
                    nc.vector.tensor_scalar(
                        invb_i, invb_i, -1, RSQRT_MAGIC, op0=ALU.mult, op1=ALU.add)
                    nr_t = nrpool.tile([P, G4 * BLK], F32, tag="nr")
                    for _ in range(2):
                        nc.vector.scalar_tensor_tensor(
                            nr_t[:, :], invb, 1.0, invb, op0=ALU.mult, op1=ALU.mult)
                        nc.vector.scalar_tensor_tensor(
                            nr_t[:, :], nr_t[:, :], -0.5, n2b, op0=ALU.mult, op1=ALU.mult)
                        nc.vector.scalar_tensor_tensor(
                            invb, nr_t[:, :], 1.5, invb, op0=ALU.add, op1=ALU.mult)

                    # normalize the G4 blocks in place, then store
                    for bb_ in range(b - G4 + 1, b + 1):
                        Fb = fblks.pop(bb_)
                        for i in range(BLK):
                            t = bb_ * BLK + i
                            nc.vector.tensor_scalar_mul(
                                Fb[:, i * D:(i + 1) * D],
                                Fb[:, i * D:(i + 1) * D],
                                inv_all[:, t:t + 1],
                            )
                        nc.scalar.dma_start(
                            onrm_v[bb_ * BLK:(bb_ + 1) * BLK].rearrange("i p d -> p i d"),
                            Fb[:, :].rearrange("p (i d) -> p i d", d=D),
                        )

            # ---- tail: cross-core allreduce + final scores ----
            dSB = tailpool.tile([NSEG, 1], F32, tag="dSB")
            nc.vector.tensor_copy(dSB[:, :], psD[:, :])
            nc.sync.dma_start(cc_in[:, :], dSB[:, :])
            nc.gpsimd.collective_compute(
                "AllReduce", ALU.add,
                replica_groups=[list(range(NCORES))],
                ins=[cc_in[:, :]],
                outs=[cc_out[:, :]],
            )
            dAll = tailpool.tile([NSEG, 1], F32, tag="dAll")
            nc.sync.dma_start(dAll[:, :], cc_out[:, :])

            # bias = -ln(denom), split hi/lo bf16 for the broadcast matmul
            lnd = tailpool.tile([NSEG, 1], F32, tag="lnd")
            nc.scalar.activation(lnd[:, :], dAll[:, :], AF.Ln)
            nc.vector.tensor_scalar_mul(lnd[:, :], lnd[:, :], -1.0)
            lh16 = tailpool.tile([NSEG, 1], BF16, tag="lh16")
            nc.vector.tensor_copy(lh16[:, :], lnd[:, :])
            lh32 = tailpool.tile([NSEG, 1], F32, tag="lh32")
            nc.vector.tensor_copy(lh32[:, :], lh16[:, :])
            llo = tailpool.tile([NSEG, 1], F32, tag="llo")
            nc.vector.tensor_tensor(llo[:, :], lnd[:, :], lh32[:, :], op=ALU.subtract)
            llo16 = tailpool.tile([NSEG, 1], BF16, tag="llo16")
            nc.vector.tensor_copy(llo16[:, :], llo[:, :])

            # broadcast bias to every row: psBias[:, 2t:2t+2] = RT_t.T @ [-lnd_hi | -lnd_lo]
            lhl = tailpool.tile([NSEG, 2], BF16, tag="lhl")
            nc.vector.tensor_copy(lhl[:, 0:1], lh16[:, :])
            nc.vector.tensor_copy(lhl[:, 1:2], llo16[:, :])
            psBias = pstail_pool.tile([P, 512], F32, tag="psBias")
            for g in range(T // RCH):
                RT_sb = rpool.tile([NSEG, RCH * P], BF16, tag="RT")
                nc.sync.dma_start(RT_sb[:, :], RTm[:, g * RCH * P:(g + 1) * RCH * P])
                for j in range(RCH):
                    t = g * RCH + j
                    rt = RT_sb[:, j * P:(j + 1) * P]
                    nc.tensor.matmul(
                        psBias[:, 2 * t:2 * t + 2], rt, lhl[:, :],
                        start=(t == 0), stop=(t == T - 1),
                    )

            s2 = tailpool.tile([P, T], F32, tag="s2")
            pbv = psBias[:, 0:2 * T].rearrange("p (t two) -> p t two", two=2)
            nc.vector.tensor_tensor(s2[:, :], s_all[:, :], pbv[:, :, 0], op=ALU.add)
            nc.vector.tensor_tensor(s2[:, :], s2[:, :], pbv[:, :, 1], op=ALU.add)
            exF = tailpool.tile([P, T], F32, tag="exF")
            nc.scalar.activation(exF[:, :], s2[:, :], AF.Exp)
            nc.sync.dma_start(out_score[:, :], exF[:, :])

    return nc


# ---------------------------------------------------------------------------
# host glue
# ---------------------------------------------------------------------------

_BUILD_CACHE: dict[int, bass.Bass] = {}


def _get_nc(rpc: int) -> bass.Bass:
    if rpc not in _BUILD_CACHE:
        nc = build(rpc)
        nc.finalize()
        _BUILD_CACHE[rpc] = nc
    return _BUILD_CACHE[rpc]


def _prep_in_maps(feature, batch, Wa, ba, Wb, bb, Wc, bc, rpc):
    T = rpc // P
    # chunk c of Wa.T is Wa.T[c*128:(c+1)*128, :] laid at cols [c*128, (c+1)*128)
    waT = np.concatenate([Wa.T[c * P:(c + 1) * P, :] for c in range(8)], axis=1)
    wbT = np.concatenate([Wb.T[c * P:(c + 1) * P, :] for c in range(8)], axis=1)

    import ml_dtypes
    tobf = lambda x: np.asarray(x, dtype=np.float32).astype(ml_dtypes.bfloat16)

    wc4 = np.broadcast_to(np.tile(np.asarray(Wc, np.float32).reshape(1, L), (1, 4)), (P, 4 * L))
    bias_ab = np.concatenate(
        [np.asarray(ba, np.float32).reshape(1, L),
         np.asarray(bb, np.float32).reshape(1, L)], axis=1)
    ones1 = np.ones((1, L), np.float32)
    ident = np.eye(P, dtype=np.float32)

    in_maps = []
    for ci in range(NCORES):
        rows = slice(ci * rpc, (ci + 1) * rpc)
        bseg = np.asarray(batch[rows], np.int64)
        # one-hot [rpc, NSEG]
        oh = np.zeros((rpc, NSEG), np.float32)
        oh[np.arange(rpc), bseg] = 1.0
        # Rm [P, T*NSEG]: Rm[p, t*NSEG+s] = oh[t*P+p, s]
        Rm = oh.reshape(T, P, NSEG).transpose(1, 0, 2).reshape(P, T * NSEG)
        # RTm [NSEG, T*P]: RTm[s, t*P+p] = oh[t*P+p, s]
        RTm = oh.T.copy()
        in_maps.append({
            "feat": np.ascontiguousarray(feature[rows]).astype(np.float32),
            "waT": tobf(waT),
            "wbT": tobf(wbT),
            "wc4": tobf(wc4),
            "bias_ab": tobf(bias_ab),
            "ones1": tobf(ones1),
            "ident": tobf(ident),
            "Rm": tobf(Rm),
            "RTm": tobf(RTm),
        })
    return in_maps


def kernel(feature, batch, istrain, Wa, ba, Wb, bb, Wc, bc):
    feature = np.asarray(feature, np.float32)
    batch_np = np.asarray(batch)
    Wa = np.asarray(Wa, np.float32)
    ba = np.asarray(ba, np.float32)
    Wb = np.asarray(Wb, np.float32)
    bb = np.asarray(bb, np.float32)
    Wc = np.asarray(Wc, np.float32)
    bc = np.asarray(bc, np.float32)

    n = feature.shape[0]
    rpc = n // NCORES
    nc = _get_nc(rpc)
    in_maps = _prep_in_maps(feature, batch_np, Wa, ba, Wb, bb, Wc, bc, rpc)

    trace = os.environ.get("KER_TRACE", "0") == "1"
    kwargs = {}
    if trace:
        kwargs["trace"] = True
        kwargs["tmpdir"] = tempfile.mkdtemp(prefix="ker_trace_")
    res = run_bass_kernel_spmd(nc, in_maps, core_ids=list(range(NCORES)), **kwargs)
    if trace:
        print(f"[kernel] exec_time_ns: {res.exec_time_ns}")
        print(f"[kernel] mean_exec_time_ns: {res.mean_exec_time_ns}")
        print(f"[kernel] trace: {res.instructions_and_trace}")
        kernel.last_results = res

    T = rpc // P
    out_norm = np.empty((n, D), np.float32)
    score = np.empty((n,), np.float32)
    for ci in range(NCORES):
        rows = slice(ci * rpc, (ci + 1) * rpc)
        out_norm[rows] = res.results[ci]["out_norm"]
        score[rows] = res.results[ci]["out_score"].T.reshape(-1)
    return out_norm, score.reshape(n, 1)


if __name__ == "__main__":
    # tiny smoke test with random data
    rng = np.random.default_rng(0)
    n_small = int(os.environ.get("SMOKE_N", 8 * 2048))
    feature = rng.standard_normal((n_small, D), dtype=np.float32)
    batch = np.sort(rng.integers(0, NSEG, n_small).astype(np.int32))
    Wa = (rng.standard_normal((L, D)) * np.sqrt(2.0 / (D + L))).astype(np.float32)
    Wb = (rng.standard_normal((L, D)) * np.sqrt(2.0 / (D + L))).astype(np.float32)
    Wc = (rng.standard_normal((1, L)) * np.sqrt(2.0 / (L + 1))).astype(np.float32)
    ba = np.zeros(L, np.float32)
    bb = np.zeros(L, np.float32)
    bc = np.zeros(1, np.float32)

    o1, o2 = kernel(feature, batch, 0, Wa, ba, Wb, bb, Wc, bc)

    # numpy reference
    nrm = np.linalg.norm(feature, axis=1, keepdims=True)
    r1 = feature / np.maximum(nrm, 1e-12)
    a = 1.0 / (1.0 + np.exp(-(feature @ Wa.T + ba)))
    b = np.tanh(feature @ Wb.T + bb)
    s = (a * b) @ Wc.T + bc
    seg_max = np.full((NSEG, 1), -np.inf)
    np.maximum.at(seg_max, batch, s)
    ex = np.exp(s - seg_max[batch])
    den = np.zeros((NSEG, 1))
    np.add.at(den, batch, ex)
    r2 = ex / (den[batch] + 1e-16)

    e1 = np.abs(o1 - r1) / (np.abs(r1) + 1e-5)
    e2 = np.abs(o2 - r2) / (np.abs(r2) + 1e-5)
    print("norm out: max rel err", e1.max(), "mean", e1.mean())
    print("score out: max rel err", e2.max(), "mean", e2.mean())


# revision 18
# speedup vs baseline: 1.1885x; 1.1885x over previous
"""Trainium2 Bass kernel for nn_Attn_Net_Gated (segment_reduce).

Computes, for feature [N, D] fp32 and sorted segment ids batch [N]:
  out1 = feature / max(||feature||_row, 1e-12)
  s    = (sigmoid(feature @ Wa.T + ba) * tanh(feature @ Wb.T + bb)) @ Wc.T + bc
  out2 = segment_softmax(s, batch)        (64 segments)

Sharding: rows split equally across 8 NeuronCores; segment denominators are
combined with a tiny [64]-float AllReduce, so segments may straddle shards.
"""

import os
import sys
import tempfile

import numpy as np

sys.path.insert(0, "/opt/trn_rl_repo")

import concourse.bass as bass
import concourse.bacc as bacc
import concourse.mybir as mybir
from concourse.tile import TileContext
from concourse.bass_utils import run_bass_kernel_spmd

N, D, L, NSEG, NCORES = 262144, 1024, 128, 64, 8
P = 128           # partitions
BLK = 4           # row-tiles per DMA block
F32 = mybir.dt.float32
BF16 = mybir.dt.bfloat16
I32 = mybir.dt.int32
AF = mybir.ActivationFunctionType
ALU = mybir.AluOpType

RSQRT_MAGIC = 0x5F3759DF


def build(rpc: int) -> bass.Bass:
    """Build the SPMD program for one core processing `rpc` rows."""
    T = rpc // P                    # row-tiles per core
    NB = T // BLK                   # DMA blocks per core
    G4 = 2                          # blocks per rsqrt batch
    assert T % BLK == 0 and NB % G4 == 0

    nc = bacc.Bacc(num_devices=NCORES)

    feat = nc.declare_dram_parameter("feat", [rpc, D], F32, isOutput=False)
    waT = nc.declare_dram_parameter("waT", [P, D], BF16, isOutput=False)
    wbT = nc.declare_dram_parameter("wbT", [P, D], BF16, isOutput=False)
    wc4 = nc.declare_dram_parameter("wc4", [P, BLK * L], BF16, isOutput=False)
    bias_ab = nc.declare_dram_parameter("bias_ab", [1, 2 * L], BF16, isOutput=False)
    ones1 = nc.declare_dram_parameter("ones1", [1, L], BF16, isOutput=False)
    ident = nc.declare_dram_parameter("ident", [P, P], BF16, isOutput=False)
    Rm = nc.declare_dram_parameter("Rm", [P, T * NSEG], BF16, isOutput=False)
    RTm = nc.declare_dram_parameter("RTm", [NSEG, T * P], BF16, isOutput=False)

    out_norm = nc.declare_dram_parameter("out_norm", [rpc, D], F32, isOutput=True)
    out_score = nc.declare_dram_parameter("out_score", [P, T], F32, isOutput=True)

    feat_v = feat[:, :].rearrange("(t p) d -> t p d", p=P)
    onrm_v = out_norm[:, :].rearrange("(t p) d -> t p d", p=P)

    NCHUNK = 4 if (T % 16 == 0 and (T // BLK) % 4 == 0) else 1
    CH = T // NCHUNK                # row-tiles per denominator-sum chunk
    RTCH = T // 8 if T % 8 == 0 else T  # row-tiles per RT chunk in the tail
    with TileContext(nc) as tc:
        with (
            tc.tile_pool(name="const", bufs=1) as cpool,
            tc.tile_pool(name="fpool", bufs=6) as fpool,
            tc.tile_pool(name="f16pool", bufs=2) as f16pool,
            tc.tile_pool(name="ftpool", bufs=2) as ftpool,
            tc.tile_pool(name="scrpool", bufs=2) as scrpool,
            tc.tile_pool(name="actpool", bufs=2) as actpool,
            tc.tile_pool(name="nrpool", bufs=2) as nrpool,
            tc.tile_pool(name="tailpool", bufs=1) as tailpool,
            tc.tile_pool(name="rpool", bufs=2) as rpool,
            tc.tile_pool(name="pstr", bufs=2, space="PSUM") as pstr_pool,
            tc.tile_pool(name="psmm", bufs=2, space="PSUM") as psmm_pool,
            tc.tile_pool(name="pstail", bufs=1, space="PSUM") as pstail_pool,
        ):
            # ---- resident constants ----
            waT_sb = cpool.tile([P, D], BF16, tag="waT")
            nc.sync.dma_start(waT_sb[:, :], waT[:, :])
            wbT_sb = cpool.tile([P, D], BF16, tag="wbT")
            nc.sync.dma_start(wbT_sb[:, :], wbT[:, :])
            wc4_sb = cpool.tile([P, BLK * L], BF16, tag="wc4")
            nc.sync.dma_start(wc4_sb[:, :], wc4[:, :])
            bias_sb = cpool.tile([1, 2 * L], BF16, tag="bias")
            nc.sync.dma_start(bias_sb[:, :], bias_ab[:, :])
            ones_sb = cpool.tile([1, L], BF16, tag="ones")
            nc.sync.dma_start(ones_sb[:, :], ones1[:, :])
            ident_sb = cpool.tile([P, P], BF16, tag="ident")
            nc.sync.dma_start(ident_sb[:, :], ident[:, :])

            s_all = cpool.tile([P, T], F32, tag="s_all")
            ex16 = cpool.tile([P, T], BF16, tag="ex16")
            psD = pstail_pool.tile([NSEG, 1], F32, tag="pstail_shared")
            n2_all = cpool.tile([P, T], F32, tag="n2_all")
            inv_all = cpool.tile([P, T], F32, tag="inv_all")

            # ---- main loop over row blocks ----
            fblks = {}
            for b in range(NB):
                Fblk = fpool.tile([P, BLK * D], F32, tag="F")
                fblks[b] = Fblk
                nc.sync.dma_start(
                    Fblk[:, :].rearrange("p (i d) -> p i d", d=D),
                    feat_v[b * BLK:(b + 1) * BLK].rearrange("i p d -> p i d"),
                )
                F16 = f16pool.tile([P, BLK * D], BF16, tag="F16")
                fT16 = ftpool.tile([P, BLK * D], BF16, tag="fT16")
                a_blk = actpool.tile([P, BLK * L], BF16, tag="a")
                b_blk = actpool.tile([P, BLK * L], BF16, tag="b")

                for i in range(BLK):
                    t = b * BLK + i
                    Fi = Fblk[:, i * D:(i + 1) * D]

                    # row sum-of-squares (ACT square + accumulate)
                    sq_scr = scrpool.tile([P, D], BF16, tag="sq")
                    nc.scalar.activation(
                        sq_scr[:, :], Fi, AF.Square,
                        accum_out=n2_all[:, t:t + 1],
                    )

                    # cast fp32 -> bf16 for the matmul path
                    nc.vector.tensor_copy(F16[:, i * D:(i + 1) * D], Fi)

                    # transpose the bf16 tile: 8 PE transposes of [128,128]
                    ps_tr = pstr_pool.tile([P, D], BF16, tag="ps_tr")
                    for c in range(8):
                        nc.tensor.transpose(
                            ps_tr[:, c * P:(c + 1) * P],
                            F16[:, i * D + c * P: i * D + (c + 1) * P],
                            ident_sb[:, :],
                        )
                    nc.vector.tensor_copy(fT16[:, i * D:(i + 1) * D], ps_tr[:, :])

                    # gated-attention matmuls: psA = f @ Wa.T, psB = f @ Wb.T
                    psA = psmm_pool.tile([P, L], F32, tag="psA")
                    psB = psmm_pool.tile([P, L], F32, tag="psB")
                    for c in range(8):
                        fc = fT16[:, i * D + c * P: i * D + (c + 1) * P]
                        nc.tensor.matmul(
                            psA[:, :], fc, waT_sb[:, c * P:(c + 1) * P],
                            start=(c == 0), stop=False,
                        )
                        nc.tensor.matmul(
                            psB[:, :], fc, wbT_sb[:, c * P:(c + 1) * P],
                            start=(c == 0), stop=False,
                        )
                    # + bias row (ba | bb) via rank-1 matmul
                    nc.tensor.matmul(
                        psA[:, :], ones_sb[:, :], bias_sb[:, 0:L],
                        start=False, stop=True,
                    )
                    nc.tensor.matmul(
                        psB[:, :], ones_sb[:, :], bias_sb[:, L:2 * L],
                        start=False, stop=True,
                    )
                    nc.scalar.activation(a_blk[:, i * L:(i + 1) * L], psA[:, :], AF.Sigmoid)
                    nc.scalar.activation(b_blk[:, i * L:(i + 1) * L], psB[:, :], AF.Tanh)

                # s[r] = sum_l a*b*wc : two block-wide DVE mults,
                # then a per-tile ACT identity with accumulate
                bw_blk = actpool.tile([P, BLK * L], BF16, tag="bw")
                nc.vector.tensor_tensor(bw_blk[:, :], b_blk[:, :], wc4_sb[:, :], op=ALU.mult)
                g_blk = actpool.tile([P, BLK * L], BF16, tag="g")
                nc.vector.tensor_tensor(g_blk[:, :], a_blk[:, :], bw_blk[:, :], op=ALU.mult)
                for i in range(BLK):
                    t = b * BLK + i
                    scr_id = scrpool.tile([P, L], BF16, tag="scr_id")
                    nc.scalar.activation(
                        scr_id[:, :], g_blk[:, i * L:(i + 1) * L], AF.Identity,
                        accum_out=s_all[:, t:t + 1],
                    )

                if (b + 1) % (NB // NCHUNK) == 0:
                    # segment-denominator partial sums for the finished chunk
                    g = (b + 1) // (NB // NCHUNK) - 1
                    t0, t1 = g * CH, (g + 1) * CH
                    nc.scalar.activation(ex16[:, t0:t1], s_all[:, t0:t1], AF.Exp)
                    R_sb = rpool.tile([P, CH * NSEG], BF16, tag="R")
                    nc.sync.dma_start(
                        R_sb[:, :], Rm[:, t0 * NSEG:t1 * NSEG])
                    for j in range(CH):
                        t = t0 + j
                        nc.tensor.matmul(
                            psD[:, :],
                            R_sb[:, j * NSEG:(j + 1) * NSEG],
                            ex16[:, t:t + 1],
                            start=(t == 0), stop=(t == T - 1),
                        )

                if b % G4 == G4 - 1:
                    # rsqrt of n2 over the last G4 blocks:
                    # bit-hack seed + 2 Newton steps (rel err ~5e-6)
                    c0 = (b - G4 + 1) * BLK
                    n2b = n2_all[:, c0:c0 + G4 * BLK]
                    invb = inv_all[:, c0:c0 + G4 * BLK]
                    invb_i = invb.bitcast(I32)
                    nc.vector.tensor_scalar(
                        invb_i, n2b.bitcast(I32), 1, None, op0=ALU.logical_shift_right)
                    nc.vector.tensor_scalar(
                        invb_i, invb_i, -1, RSQRT_MAGIC, op0=ALU.mult, op1=ALU.add)
                    nr_t = nrpool.tile([P, G4 * BLK], F32, tag="nr")
                    for _ in range(2):
                        nc.vector.scalar_tensor_tensor(
                            nr_t[:, :], invb, 1.0, invb, op0=ALU.mult, op1=ALU.mult)
                        nc.vector.scalar_tensor_tensor(
                            nr_t[:, :], nr_t[:, :], -0.5, n2b, op0=ALU.mult, op1=ALU.mult)
                        nc.vector.scalar_tensor_tensor(
                            invb, nr_t[:, :], 1.5, invb, op0=ALU.add, op1=ALU.mult)

                    # normalize the G4 blocks in place, then store
                    for bb_ in range(b - G4 + 1, b + 1):
                        Fb = fblks.pop(bb_)
                        for i in range(BLK):
                            t = bb_ * BLK + i
                            nc.vector.tensor_scalar_mul(
                                Fb[:, i * D:(i + 1) * D],
                                Fb[:, i * D:(i + 1) * D],
                                inv_all[:, t:t + 1],
                            )
                        nc.sync.dma_start(
                            onrm_v[bb_ * BLK:(bb_ + 1) * BLK].rearrange("i p d -> p i d"),
                            Fb[:, :].rearrange("p (i d) -> p i d", d=D),
                        )

            # ---- tail: local denominators (segment-aligned shards) ----
            dSB = tailpool.tile([NSEG, 1], F32, tag="dSB")
            # +1e-30 so absent segments give ln(1e-30) instead of ln(0) = -inf
            nc.vector.tensor_scalar_add(dSB[:, :], psD[:, :], 1e-30)

            # bias = -ln(denom), split hi/lo bf16 for the broadcast matmul
            lnd = tailpool.tile([NSEG, 1], F32, tag="lnd")
            nc.scalar.activation(lnd[:, :], dSB[:, :], AF.Ln)
            nc.vector.tensor_scalar_mul(lnd[:, :], lnd[:, :], -1.0)
            lh16 = tailpool.tile([NSEG, 1], BF16, tag="lh16")
            nc.vector.tensor_copy(lh16[:, :], lnd[:, :])
            lh32 = tailpool.tile([NSEG, 1], F32, tag="lh32")
            nc.vector.tensor_copy(lh32[:, :], lh16[:, :])
            llo = tailpool.tile([NSEG, 1], F32, tag="llo")
            nc.vector.tensor_tensor(llo[:, :], lnd[:, :], lh32[:, :], op=ALU.subtract)
            llo16 = tailpool.tile([NSEG, 1], BF16, tag="llo16")
            nc.vector.tensor_copy(llo16[:, :], llo[:, :])

            # broadcast bias to every row: psBias[:, 2t:2t+2] = RT_t.T @ [-lnd_hi | -lnd_lo]
            lhl = tailpool.tile([NSEG, 2], BF16, tag="lhl")
            nc.vector.tensor_copy(lhl[:, 0:1], lh16[:, :])
            nc.vector.tensor_copy(lhl[:, 1:2], llo16[:, :])
            # psBias shares the PSUM slot with psD (lifetimes are disjoint);
            # 2*T columns span two banks -> one accumulation group per bank
            psBias = pstail_pool.tile([P, 1024], F32, tag="pstail_shared")
            BANK = 512 // 2  # tiles per psum bank at 2 cols each
            for g in range(T // RTCH):
                RT_sb = rpool.tile([NSEG, RTCH * P], BF16, tag="RT")
                nc.sync.dma_start(RT_sb[:, :], RTm[:, g * RTCH * P:(g + 1) * RTCH * P])
                for j in range(RTCH):
                    t = g * RTCH + j
                    rt = RT_sb[:, j * P:(j + 1) * P]
                    nc.tensor.matmul(
                        psBias[:, 2 * t:2 * t + 2], rt, lhl[:, :],
                        start=(t % BANK == 0),
                        stop=(t == T - 1 or t % BANK == BANK - 1),
                    )

            s2 = tailpool.tile([P, T], F32, tag="s2")
            pbv = psBias[:, 0:2 * T].rearrange("p (t two) -> p t two", two=2)
            nc.vector.tensor_tensor(s2[:, :], s_all[:, :], pbv[:, :, 0], op=ALU.add)
            nc.vector.tensor_tensor(s2[:, :], s2[:, :], pbv[:, :, 1], op=ALU.add)
            exF = tailpool.tile([P, T], F32, tag="exF")
            nc.scalar.activation(exF[:, :], s2[:, :], AF.Exp)
            nc.sync.dma_start(out_score[:, :], exF[:, :])

    return nc


# ---------------------------------------------------------------------------
# host glue
# ---------------------------------------------------------------------------

_BUILD_CACHE: dict[int, bass.Bass] = {}


def _get_nc(rpc: int) -> bass.Bass:
    if rpc not in _BUILD_CACHE:
        nc = build(rpc)
        nc.finalize()
        _BUILD_CACHE[rpc] = nc
    return _BUILD_CACHE[rpc]


def _prep_in_maps(feature, batch, Wa, ba, Wb, bb, Wc, bc, rpad, bounds):
    T = rpad // P
    # chunk c of Wa.T is Wa.T[c*128:(c+1)*128, :] laid at cols [c*128, (c+1)*128)
    waT = np.concatenate([Wa.T[c * P:(c + 1) * P, :] for c in range(8)], axis=1)
    wbT = np.concatenate([Wb.T[c * P:(c + 1) * P, :] for c in range(8)], axis=1)

    import ml_dtypes
    tobf = lambda x: np.asarray(x, dtype=np.float32).astype(ml_dtypes.bfloat16)

    wc4 = np.broadcast_to(np.tile(np.asarray(Wc, np.float32).reshape(1, L), (1, 4)), (P, 4 * L))
    bias_ab = np.concatenate(
        [np.asarray(ba, np.float32).reshape(1, L),
         np.asarray(bb, np.float32).reshape(1, L)], axis=1)
    ones1 = np.ones((1, L), np.float32)
    ident = np.eye(P, dtype=np.float32)

    in_maps = []
    for ci in range(NCORES):
        r0, r1 = bounds[ci], bounds[ci + 1]
        ln = r1 - r0
        fpad = np.zeros((rpad, D), np.float32)
        fpad[:ln] = feature[r0:r1]
        # one-hot [rpad, NSEG]; padded rows select no segment
        oh = np.zeros((rpad, NSEG), np.float32)
        oh[np.arange(ln), np.asarray(batch[r0:r1], np.int64)] = 1.0
        Rm = oh.reshape(T, P, NSEG).transpose(1, 0, 2).reshape(P, T * NSEG)
        RTm = oh.T.copy()
        in_maps.append({
            "feat": fpad,
            "waT": tobf(waT),
            "wbT": tobf(wbT),
            "wc4": tobf(wc4),
            "bias_ab": tobf(bias_ab),
            "ones1": tobf(ones1),
            "ident": tobf(ident),
            "Rm": tobf(Rm),
            "RTm": tobf(RTm),
        })
    return in_maps


def kernel(feature, batch, istrain, Wa, ba, Wb, bb, Wc, bc):
    feature = np.asarray(feature, np.float32)
    batch_np = np.asarray(batch)
    Wa = np.asarray(Wa, np.float32)
    ba = np.asarray(ba, np.float32)
    Wb = np.asarray(Wb, np.float32)
    bb = np.asarray(bb, np.float32)
    Wc = np.asarray(Wc, np.float32)
    bc = np.asarray(bc, np.float32)

    n = feature.shape[0]
    # segment-aligned shard boundaries: core i owns segments [8i, 8i+8)
    counts = np.bincount(batch_np.astype(np.int64), minlength=NSEG)
    spc = NSEG // NCORES
    cum = np.concatenate([[0], np.cumsum(counts)])
    bounds = [int(cum[ci * spc]) for ci in range(NCORES)] + [n]
    maxlen = max(bounds[ci + 1] - bounds[ci] for ci in range(NCORES))
    rpad = ((maxlen + BLK * P * 4 - 1) // (BLK * P * 4)) * (BLK * P * 4)

    nc = _get_nc(rpad)
    in_maps = _prep_in_maps(feature, batch_np, Wa, ba, Wb, bb, Wc, bc, rpad, bounds)

    trace = os.environ.get("KER_TRACE", "0") == "1"
    kwargs = {}
    if trace:
        kwargs["trace"] = True
        kwargs["tmpdir"] = tempfile.mkdtemp(prefix="ker_trace_")
    res = run_bass_kernel_spmd(nc, in_maps, core_ids=list(range(NCORES)), **kwargs)
    if trace:
        print(f"[kernel] exec_time_ns: {res.exec_time_ns}")
        print(f"[kernel] mean_exec_time_ns: {res.mean_exec_time_ns}")
        kernel.last_results = res

    out_norm = np.empty((n, D), np.float32)
    score = np.empty((n,), np.float32)
    for ci in range(NCORES):
        r0, r1 = bounds[ci], bounds[ci + 1]
        ln = r1 - r0
        out_norm[r0:r1] = res.results[ci]["out_norm"][:ln]
        score[r0:r1] = res.results[ci]["out_score"].T.reshape(-1)[:ln]
    return out_norm, score.reshape(n, 1)


if __name__ == "__main__":
    # tiny smoke test with random data
    rng = np.random.default_rng(0)
    n_small = int(os.environ.get("SMOKE_N", 8 * 2048))
    feature = rng.standard_normal((n_small, D), dtype=np.float32)
    batch = np.sort(rng.integers(0, NSEG, n_small).astype(np.int32))
    Wa = (rng.standard_normal((L, D)) * np.sqrt(2.0 / (D + L))).astype(np.float32)
    Wb = (rng.standard_normal((L, D)) * np.sqrt(2.0 / (D + L))).astype(np.float32)
    Wc = (rng.standard_normal((1, L)) * np.sqrt(2.0 / (L + 1))).astype(np.float32)
    ba = np.zeros(L, np.float32)
    bb = np.zeros(L, np.float32)
    bc = np.zeros(1, np.float32)

    o1, o2 = kernel(feature, batch, 0, Wa, ba, Wb, bb, Wc, bc)

    # numpy reference
    nrm = np.linalg.norm(feature, axis=1, keepdims=True)
    r1 = feature / np.maximum(nrm, 1e-12)
    a = 1.0 / (1.0 + np.exp(-(feature @ Wa.T + ba)))
    b = np.tanh(feature @ Wb.T + bb)
    s = (a * b) @ Wc.T + bc
    seg_max = np.full((NSEG, 1), -np.inf)
    np.maximum.at(seg_max, batch, s)
    ex = np.exp(s - seg_max[batch])
    den = np.zeros((NSEG, 1))
    np.add.at(den, batch, ex)
    r2 = ex / (den[batch] + 1e-16)

    e1 = np.abs(o1 - r1) / (np.abs(r1) + 1e-5)
    e2 = np.abs(o2 - r2) / (np.abs(r2) + 1e-5)
    print("norm out: max rel err", e1.max(), "mean", e1.mean())
    print("score out: max rel err", e2.max(), "mean", e2.mean())


# revision 20
# speedup vs baseline: 1.2939x; 1.0887x over previous
"""Trainium2 Bass kernel for nn_Attn_Net_Gated (segment_reduce).

Computes, for feature [N, D] fp32 and sorted segment ids batch [N]:
  out1 = feature / max(||feature||_row, 1e-12)
  s    = (sigmoid(feature @ Wa.T + ba) * tanh(feature @ Wb.T + bb)) @ Wc.T + bc
  out2 = segment_softmax(s, batch)        (64 segments)

Sharding: rows split equally across 8 NeuronCores; segment denominators are
combined with a tiny [64]-float AllReduce, so segments may straddle shards.
"""

import os
import sys
import tempfile

import numpy as np

sys.path.insert(0, "/opt/trn_rl_repo")

import concourse.bass as bass
import concourse.bacc as bacc
import concourse.mybir as mybir
from concourse.tile import TileContext
from concourse.bass_utils import run_bass_kernel_spmd

N, D, L, NSEG, NCORES = 262144, 1024, 128, 64, 8
P = 128           # partitions
BLK = 4           # row-tiles per DMA block
F32 = mybir.dt.float32
BF16 = mybir.dt.bfloat16
I32 = mybir.dt.int32
AF = mybir.ActivationFunctionType
ALU = mybir.AluOpType

RSQRT_MAGIC = 0x5F3759DF


def build(rpc: int) -> bass.Bass:
    """Build the SPMD program for one core processing `rpc` rows."""
    T = rpc // P                    # row-tiles per core
    NB = T // BLK                   # DMA blocks per core
    G4 = 2 if NB % 2 == 0 else 1    # blocks per rsqrt batch
    assert T % BLK == 0 and NB % G4 == 0

    nc = bacc.Bacc(num_devices=NCORES)

    feat = nc.declare_dram_parameter("feat", [rpc, D], F32, isOutput=False)
    waT = nc.declare_dram_parameter("waT", [P, D], BF16, isOutput=False)
    wbT = nc.declare_dram_parameter("wbT", [P, D], BF16, isOutput=False)
    wc4 = nc.declare_dram_parameter("wc4", [P, BLK * L], BF16, isOutput=False)
    bias_ab = nc.declare_dram_parameter("bias_ab", [1, 2 * L], BF16, isOutput=False)
    ones1 = nc.declare_dram_parameter("ones1", [1, L], BF16, isOutput=False)
    ident = nc.declare_dram_parameter("ident", [P, P], BF16, isOutput=False)
    Rm = nc.declare_dram_parameter("Rm", [P, T * NSEG], BF16, isOutput=False)
    RTm = nc.declare_dram_parameter("RTm", [NSEG, T * P], BF16, isOutput=False)

    out_norm = nc.declare_dram_parameter("out_norm", [rpc, D], F32, isOutput=True)
    out_score = nc.declare_dram_parameter("out_score", [P, T], F32, isOutput=True)

    feat_v = feat[:, :].rearrange("(t p) d -> t p d", p=P)
    onrm_v = out_norm[:, :].rearrange("(t p) d -> t p d", p=P)

    NCHUNK = next(c for c in (4, 3, 2, 1) if T % c == 0 and NB % c == 0)
    CH = T // NCHUNK                # row-tiles per denominator-sum chunk
    RCH = next(c for c in range(min(52, CH), 0, -1) if CH % c == 0)
    RTCH = next(c for c in range(min(52, T), 0, -1) if T % c == 0)
    with TileContext(nc) as tc:
        with (
            tc.tile_pool(name="const", bufs=1) as cpool,
            tc.tile_pool(name="fpool", bufs=6) as fpool,
            tc.tile_pool(name="f16pool", bufs=2) as f16pool,
            tc.tile_pool(name="ftpool", bufs=2) as ftpool,
            tc.tile_pool(name="scrpool", bufs=2) as scrpool,
            tc.tile_pool(name="actpool", bufs=2) as actpool,
            tc.tile_pool(name="nrpool", bufs=2) as nrpool,
            tc.tile_pool(name="tailpool", bufs=1) as tailpool,
            tc.tile_pool(name="rpool", bufs=2) as rpool,
            tc.tile_pool(name="pstr", bufs=2, space="PSUM") as pstr_pool,
            tc.tile_pool(name="psmm", bufs=2, space="PSUM") as psmm_pool,
            tc.tile_pool(name="pstail", bufs=1, space="PSUM") as pstail_pool,
        ):
            # ---- resident constants ----
            waT_sb = cpool.tile([P, D], BF16, tag="waT")
            nc.sync.dma_start(waT_sb[:, :], waT[:, :])
            wbT_sb = cpool.tile([P, D], BF16, tag="wbT")
            nc.sync.dma_start(wbT_sb[:, :], wbT[:, :])
            wc4_sb = cpool.tile([P, BLK * L], BF16, tag="wc4")
            nc.sync.dma_start(wc4_sb[:, :], wc4[:, :])
            bias_sb = cpool.tile([1, 2 * L], BF16, tag="bias")
            nc.sync.dma_start(bias_sb[:, :], bias_ab[:, :])
            ones_sb = cpool.tile([1, L], BF16, tag="ones")
            nc.sync.dma_start(ones_sb[:, :], ones1[:, :])
            ident_sb = cpool.tile([P, P], BF16, tag="ident")
            nc.sync.dma_start(ident_sb[:, :], ident[:, :])

            s_all = cpool.tile([P, T], F32, tag="s_all")
            ex16 = cpool.tile([P, T], BF16, tag="ex16")
            psD = pstail_pool.tile([NSEG, 1], F32, tag="pstail_shared")
            n2_all = cpool.tile([P, T], F32, tag="n2_all")
            inv_all = cpool.tile([P, T], F32, tag="inv_all")

            # ---- main loop over row blocks ----
            fblks = {}
            for b in range(NB):
                Fblk = fpool.tile([P, BLK * D], F32, tag="F")
                fblks[b] = Fblk
                nc.sync.dma_start(
                    Fblk[:, :].rearrange("p (i d) -> p i d", d=D),
                    feat_v[b * BLK:(b + 1) * BLK].rearrange("i p d -> p i d"),
                )
                F16 = f16pool.tile([P, BLK * D], BF16, tag="F16")
                fT16 = ftpool.tile([P, BLK * D], BF16, tag="fT16")
                a_blk = actpool.tile([P, BLK * L], BF16, tag="a")
                b_blk = actpool.tile([P, BLK * L], BF16, tag="b")

                for i in range(BLK):
                    t = b * BLK + i
                    Fi = Fblk[:, i * D:(i + 1) * D]

                    # row sum-of-squares (ACT square + accumulate)
                    sq_scr = scrpool.tile([P, D], BF16, tag="sq")
                    nc.scalar.activation(
                        sq_scr[:, :], Fi, AF.Square,
                        accum_out=n2_all[:, t:t + 1],
                    )

                    # cast fp32 -> bf16 for the matmul path
                    nc.vector.tensor_copy(F16[:, i * D:(i + 1) * D], Fi)

                    # transpose the bf16 tile: 8 PE transposes of [128,128]
                    ps_tr = pstr_pool.tile([P, D], BF16, tag="ps_tr")
                    for c in range(8):
                        nc.tensor.transpose(
                            ps_tr[:, c * P:(c + 1) * P],
                            F16[:, i * D + c * P: i * D + (c + 1) * P],
                            ident_sb[:, :],
                        )
                    nc.vector.tensor_copy(fT16[:, i * D:(i + 1) * D], ps_tr[:, :])

                    # gated-attention matmuls: psA = f @ Wa.T, psB = f @ Wb.T
                    psA = psmm_pool.tile([P, L], F32, tag="psA")
                    psB = psmm_pool.tile([P, L], F32, tag="psB")
                    for c in range(8):
                        fc = fT16[:, i * D + c * P: i * D + (c + 1) * P]
                        nc.tensor.matmul(
                            psA[:, :], fc, waT_sb[:, c * P:(c + 1) * P],
                            start=(c == 0), stop=False,
                        )
                        nc.tensor.matmul(
                            psB[:, :], fc, wbT_sb[:, c * P:(c + 1) * P],
                            start=(c == 0), stop=False,
                        )
                    # + bias row (ba | bb) via rank-1 matmul
                    nc.tensor.matmul(
                        psA[:, :], ones_sb[:, :], bias_sb[:, 0:L],
                        start=False, stop=True,
                    )
                    nc.tensor.matmul(
                        psB[:, :], ones_sb[:, :], bias_sb[:, L:2 * L],
                        start=False, stop=True,
                    )
                    nc.scalar.activation(a_blk[:, i * L:(i + 1) * L], psA[:, :], AF.Sigmoid)
                    nc.scalar.activation(b_blk[:, i * L:(i + 1) * L], psB[:, :], AF.Tanh)

                # s[r] = sum_l a*b*wc : two block-wide DVE mults,
                # then a per-tile ACT identity with accumulate
                bw_blk = actpool.tile([P, BLK * L], BF16, tag="bw")
                nc.vector.tensor_tensor(bw_blk[:, :], b_blk[:, :], wc4_sb[:, :], op=ALU.mult)
                g_blk = actpool.tile([P, BLK * L], BF16, tag="g")
                nc.vector.tensor_tensor(g_blk[:, :], a_blk[:, :], bw_blk[:, :], op=ALU.mult)
                for i in range(BLK):
                    t = b * BLK + i
                    scr_id = scrpool.tile([P, L], BF16, tag="scr_id")
                    nc.scalar.activation(
                        scr_id[:, :], g_blk[:, i * L:(i + 1) * L], AF.Identity,
                        accum_out=s_all[:, t:t + 1],
                    )

                if (b + 1) % (NB // NCHUNK) == 0:
                    # segment-denominator partial sums for the finished chunk
                    g = (b + 1) // (NB // NCHUNK) - 1
                    t0, t1 = g * CH, (g + 1) * CH
                    nc.scalar.activation(ex16[:, t0:t1], s_all[:, t0:t1], AF.Exp)
                    for rg in range(CH // RCH):
                        r0 = t0 + rg * RCH
                        R_sb = rpool.tile([P, RCH * NSEG], BF16, tag="R")
                        nc.sync.dma_start(
                            R_sb[:, :], Rm[:, r0 * NSEG:(r0 + RCH) * NSEG])
                        for j in range(RCH):
                            t = r0 + j
                            nc.tensor.matmul(
                                psD[:, :],
                                R_sb[:, j * NSEG:(j + 1) * NSEG],
                                ex16[:, t:t + 1],
                                start=(t == 0), stop=(t == T - 1),
                            )

                if b % G4 == G4 - 1:
                    # rsqrt of n2 over the last G4 blocks:
                    # bit-hack seed + 2 Newton steps (rel err ~5e-6)
                    c0 = (b - G4 + 1) * BLK
                    n2b = n2_all[:, c0:c0 + G4 * BLK]
                    invb = inv_all[:, c0:c0 + G4 * BLK]
                    invb_i = invb.bitcast(I32)
                    nc.vector.tensor_scalar(
                        invb_i, n2b.bitcast(I32), 1, None, op0=ALU.logical_shift_right)
                    nc.vector.tensor_scalar(
                        invb_i, invb_i, -1, RSQRT_MAGIC, op0=ALU.mult, op1=ALU.add)
                    nr_t = nrpool.tile([P, G4 * BLK], F32, tag="nr")
                    for _ in range(2):
                        nc.vector.scalar_tensor_tensor(
                            nr_t[:, :], invb, 1.0, invb, op0=ALU.mult, op1=ALU.mult)
                        nc.vector.scalar_tensor_tensor(
                            nr_t[:, :], nr_t[:, :], -0.5, n2b, op0=ALU.mult, op1=ALU.mult)
                        nc.vector.scalar_tensor_tensor(
                            invb, nr_t[:, :], 1.5, invb, op0=ALU.add, op1=ALU.mult)

                    # normalize the G4 blocks in place, then store
                    for bb_ in range(b - G4 + 1, b + 1):
                        Fb = fblks.pop(bb_)
                        for i in range(BLK):
                            t = bb_ * BLK + i
                            nc.vector.tensor_scalar_mul(
                                Fb[:, i * D:(i + 1) * D],
                                Fb[:, i * D:(i + 1) * D],
                                inv_all[:, t:t + 1],
                            )
                        nc.sync.dma_start(
                            onrm_v[bb_ * BLK:(bb_ + 1) * BLK].rearrange("i p d -> p i d"),
                            Fb[:, :].rearrange("p (i d) -> p i d", d=D),
                        )

            # ---- tail: local denominators (segment-aligned shards) ----
            dSB = tailpool.tile([NSEG, 1], F32, tag="dSB")
            # +1e-30 so absent segments give ln(1e-30) instead of ln(0) = -inf
            nc.vector.tensor_scalar_add(dSB[:, :], psD[:, :], 1e-30)

            # bias = -ln(denom), split hi/lo bf16 for the broadcast matmul
            lnd = tailpool.tile([NSEG, 1], F32, tag="lnd")
            nc.scalar.activation(lnd[:, :], dSB[:, :], AF.Ln)
            nc.vector.tensor_scalar_mul(lnd[:, :], lnd[:, :], -1.0)
            lh16 = tailpool.tile([NSEG, 1], BF16, tag="lh16")
            nc.vector.tensor_copy(lh16[:, :], lnd[:, :])
            lh32 = tailpool.tile([NSEG, 1], F32, tag="lh32")
            nc.vector.tensor_copy(lh32[:, :], lh16[:, :])
            llo = tailpool.tile([NSEG, 1], F32, tag="llo")
            nc.vector.tensor_tensor(llo[:, :], lnd[:, :], lh32[:, :], op=ALU.subtract)
            llo16 = tailpool.tile([NSEG, 1], BF16, tag="llo16")
            nc.vector.tensor_copy(llo16[:, :], llo[:, :])

            # broadcast bias to every row: psBias[:, 2t:2t+2] = RT_t.T @ [-lnd_hi | -lnd_lo]
            lhl = tailpool.tile([NSEG, 2], BF16, tag="lhl")
            nc.vector.tensor_copy(lhl[:, 0:1], lh16[:, :])
            nc.vector.tensor_copy(lhl[:, 1:2], llo16[:, :])
            # psBias shares the PSUM slot with psD (lifetimes are disjoint);
            # 2*T columns span two banks -> one accumulation group per bank
            psBias = pstail_pool.tile([P, 1024], F32, tag="pstail_shared")
            BANK = 512 // 2  # tiles per psum bank at 2 cols each
            for g in range(T // RTCH):
                RT_sb = rpool.tile([NSEG, RTCH * P], BF16, tag="RT")
                nc.sync.dma_start(RT_sb[:, :], RTm[:, g * RTCH * P:(g + 1) * RTCH * P])
                for j in range(RTCH):
                    t = g * RTCH + j
                    rt = RT_sb[:, j * P:(j + 1) * P]
                    nc.tensor.matmul(
                        psBias[:, 2 * t:2 * t + 2], rt, lhl[:, :],
                        start=(t % BANK == 0),
                        stop=(t == T - 1 or t % BANK == BANK - 1),
                    )

            s2 = tailpool.tile([P, T], F32, tag="s2")
            pbv = psBias[:, 0:2 * T].rearrange("p (t two) -> p t two", two=2)
            nc.vector.tensor_tensor(s2[:, :], s_all[:, :], pbv[:, :, 0], op=ALU.add)
            nc.vector.tensor_tensor(s2[:, :], s2[:, :], pbv[:, :, 1], op=ALU.add)
            exF = tailpool.tile([P, T], F32, tag="exF")
            nc.scalar.activation(exF[:, :], s2[:, :], AF.Exp)
            nc.sync.dma_start(out_score[:, :], exF[:, :])

    return nc


# ---------------------------------------------------------------------------
# host glue
# ---------------------------------------------------------------------------

_BUILD_CACHE: dict[int, bass.Bass] = {}


def _get_nc(rpc: int) -> bass.Bass:
    if rpc not in _BUILD_CACHE:
        nc = build(rpc)
        nc.finalize()
        _BUILD_CACHE[rpc] = nc
    return _BUILD_CACHE[rpc]


def _prep_in_maps(feature, batch, Wa, ba, Wb, bb, Wc, bc, rpad, bounds):
    T = rpad // P
    # chunk c of Wa.T is Wa.T[c*128:(c+1)*128, :] laid at cols [c*128, (c+1)*128)
    waT = np.concatenate([Wa.T[c * P:(c + 1) * P, :] for c in range(8)], axis=1)
    wbT = np.concatenate([Wb.T[c * P:(c + 1) * P, :] for c in range(8)], axis=1)

    import ml_dtypes
    tobf = lambda x: np.asarray(x, dtype=np.float32).astype(ml_dtypes.bfloat16)

    wc4 = np.broadcast_to(np.tile(np.asarray(Wc, np.float32).reshape(1, L), (1, 4)), (P, 4 * L))
    bias_ab = np.concatenate(
        [np.asarray(ba, np.float32).reshape(1, L),
         np.asarray(bb, np.float32).reshape(1, L)], axis=1)
    ones1 = np.ones((1, L), np.float32)
    ident = np.eye(P, dtype=np.float32)

    in_maps = []
    for ci in range(NCORES):
        r0, r1 = bounds[ci], bounds[ci + 1]
        ln = r1 - r0
        fpad = np.zeros((rpad, D), np.float32)
        fpad[:ln] = feature[r0:r1]
        # one-hot [rpad, NSEG]; padded rows select no segment
        oh = np.zeros((rpad, NSEG), np.float32)
        oh[np.arange(ln), np.asarray(batch[r0:r1], np.int64)] = 1.0
        Rm = oh.reshape(T, P, NSEG).transpose(1, 0, 2).reshape(P, T * NSEG)
        RTm = oh.T.copy()
        in_maps.append({
            "feat": fpad,
            "waT": tobf(waT),
            "wbT": tobf(wbT),
            "wc4": tobf(wc4),
            "bias_ab": tobf(bias_ab),
            "ones1": tobf(ones1),
            "ident": tobf(ident),
            "Rm": tobf(Rm),
            "RTm": tobf(RTm),
        })
    return in_maps


def kernel(feature, batch, istrain, Wa, ba, Wb, bb, Wc, bc):
    feature = np.asarray(feature, np.float32)
    batch_np = np.asarray(batch)
    Wa = np.asarray(Wa, np.float32)
    ba = np.asarray(ba, np.float32)
    Wb = np.asarray(Wb, np.float32)
    bb = np.asarray(bb, np.float32)
    Wc = np.asarray(Wc, np.float32)
    bc = np.asarray(bc, np.float32)

    n = feature.shape[0]
    # segment-aligned shards: contiguous segment ranges per core, chosen by
    # DP to minimize the max shard length (shards are padded to that max)
    counts = np.bincount(batch_np.astype(np.int64), minlength=NSEG)
    cum = np.concatenate([[0], np.cumsum(counts)])  # cum[s] = rows before seg s

    def seg_partition(cum, k):
        nseg = len(cum) - 1
        INF = float("inf")
        best = [[INF] * (k + 1) for _ in range(nseg + 1)]
        cut = [[0] * (k + 1) for _ in range(nseg + 1)]
        best[0][0] = 0.0
        for j in range(1, k + 1):
            for s in range(1, nseg + 1):
                for s0 in range(j - 1, s):
                    v = max(best[s0][j - 1], cum[s] - cum[s0])
                    if v < best[s][j]:
                        best[s][j] = v
                        cut[s][j] = s0
        segs = [nseg]
        for j in range(k, 0, -1):
            segs.append(cut[segs[-1]][j])
        return segs[::-1]

    segcuts = seg_partition(cum, NCORES)
    bounds = [int(cum[s]) for s in segcuts]
    maxlen = max(bounds[ci + 1] - bounds[ci] for ci in range(NCORES))
    rpad = ((maxlen + BLK * P - 1) // (BLK * P)) * (BLK * P)

    nc = _get_nc(rpad)
    in_maps = _prep_in_maps(feature, batch_np, Wa, ba, Wb, bb, Wc, bc, rpad, bounds)

    trace = os.environ.get("KER_TRACE", "0") == "1"
    kwargs = {}
    if trace:
        kwargs["trace"] = True
        kwargs["tmpdir"] = tempfile.mkdtemp(prefix="ker_trace_")
    res = run_bass_kernel_spmd(nc, in_maps, core_ids=list(range(NCORES)), **kwargs)
    if trace:
        print(f"[kernel] exec_time_ns: {res.exec_time_ns}")
        print(f"[kernel] mean_exec_time_ns: {res.mean_exec_time_ns}")
        kernel.last_results = res

    out_norm = np.empty((n, D), np.float32)
    score = np.empty((n,), np.float32)
    for ci in range(NCORES):
        r0, r1 = bounds[ci], bounds[ci + 1]
        ln = r1 - r0
        out_norm[r0:r1] = res.results[ci]["out_norm"][:ln]
        score[r0:r1] = res.results[ci]["out_score"].T.reshape(-1)[:ln]
    return out_norm, score.reshape(n, 1)


if __name__ == "__main__":
    # tiny smoke test with random data
    rng = np.random.default_rng(0)
    n_small = int(os.environ.get("SMOKE_N", 8 * 2048))
    feature = rng.standard_normal((n_small, D), dtype=np.float32)
    batch = np.sort(rng.integers(0, NSEG, n_small).astype(np.int32))
    Wa = (rng.standard_normal((L, D)) * np.sqrt(2.0 / (D + L))).astype(np.float32)
    Wb = (rng.standard_normal((L, D)) * np.sqrt(2.0 / (D + L))).astype(np.float32)
    Wc = (rng.standard_normal((1, L)) * np.sqrt(2.0 / (L + 1))).astype(np.float32)
    ba = np.zeros(L, np.float32)
    bb = np.zeros(L, np.float32)
    bc = np.zeros(1, np.float32)

    o1, o2 = kernel(feature, batch, 0, Wa, ba, Wb, bb, Wc, bc)

    # numpy reference
    nrm = np.linalg.norm(feature, axis=1, keepdims=True)
    r1 = feature / np.maximum(nrm, 1e-12)
    a = 1.0 / (1.0 + np.exp(-(feature @ Wa.T + ba)))
    b = np.tanh(feature @ Wb.T + bb)
    s = (a * b) @ Wc.T + bc
    seg_max = np.full((NSEG, 1), -np.inf)
    np.maximum.at(seg_max, batch, s)
    ex = np.exp(s - seg_max[batch])
    den = np.zeros((NSEG, 1))
    np.add.at(den, batch, ex)
    r2 = ex / (den[batch] + 1e-16)

    e1 = np.abs(o1 - r1) / (np.abs(r1) + 1e-5)
    e2 = np.abs(o2 - r2) / (np.abs(r2) + 1e-5)
    print("norm out: max rel err", e1.max(), "mean", e1.mean())
    print("score out: max rel err", e2.max(), "mean", e2.mean())
